# revision 1
# baseline (speedup 1.0000x reference)
"""Brownian-bridge criterion loss on 8 Trainium2 NeuronCores.

Strategy (data-parallel over the n = bs*q sequence axis, hint-compliant):
  Host-side (indexing only): sort sequences by bridge pivot, shard 200
  cur + 200 other sequences per core.

  Launch 1 (per core): project+bias its 6400 rows (seq*t) with W,b,
  L2-normalize, write embeddings; compute per-row a, self-dist s,
  numer, c0/c1 = dist-affine coeffs, softplus(head-tail) terms.

  Host reshard (indexing only): build the pivot-grouped padded A matrix
  (256 x 3584 slots, group capacity 256) and each core's negative-pool
  slice (its own 400 sequences at the 14 pivot positions).

  Launch 2 (per core, column-sharded cross matmul): for each of 28
  M-tiles (pivot group g = m//2), cross = A_g.T @ pool_g -> (128, 400)
  PSUM, then a single Max8 per tile gives that core's top-8 candidate
  cross values per row. No masking needed: self-exclusion is handled
  arithmetically in launch 3.

  Host gather (indexing only): per sorted row, concat the 8 cores'
  top-8 raw cross values -> (1600, 64).

  Launch 3 (replicated): dist = c1*cross + c0 (monotone per row, so
  top-k commutes with the affine), top-8 of 64, then
  sum(exp(top5 excl self)) = sum_{i<=6} exp(v_i) - exp(max(s, v6)),
  deno = numer + that, loss_i = numer/deno; means via ones-matmul.

The kernel structure is value-independent: bridge contents only change
index/input tensors, never shapes or instruction streams.
"""

import sys

sys.path.insert(0, "/opt/trn_rl_repo")

import numpy as np

import concourse.bacc as bacc
import concourse.bass as bass
import concourse.mybir as mybir
import concourse.tile as tile
from concourse.bass_utils import run_bass_kernel_spmd

F32 = mybir.dt.float32
F32R = mybir.dt.float32r
I32 = mybir.dt.int32
AF = mybir.ActivationFunctionType
OP = mybir.AluOpType

BS, T, Q, HID, PROJ = 16, 16, 100, 256, 256
NSEQ = BS * Q              # 1600 positive sequences
NCORES = 8
SPC = NSEQ // NCORES       # 200 cur sequences per core
RPC = 2 * SPC * T          # 6400 projected rows per core (cur+other)
NG = T - 2                 # 14 pivot groups (pivot in 1..14)
GCAP = 256                 # padded slots per pivot group (max count ~140)
SLOTS = NG * GCAP          # 3584
NCOL = 2 * SPC             # 400 negative-pool columns per core
ROWT3 = (NSEQ + 127) // 128  # 13 row tiles in launch 3
NPAD3 = ROWT3 * 128        # 1664
DELTA = 0.3

MM_DT = F32R               # matmul operand dtype (F32R: full-rate, ~tf32)
KAUG = HID + 1             # 257: contraction with bias row folded in


def _seq_tiles():
    """Partition tiles covering SPC sequences: [(start, size), ...]."""
    out = []
    s = 0
    while s < SPC:
        out.append((s, min(128, SPC - s)))
        s += 128
    return out


def _build_l1():
    nc = bacc.Bacc("TRN2", target_bir_lowering=False, debug=False,
                   num_devices=NCORES)
    xt_cur = nc.declare_dram_parameter("xt_cur", [HID, SPC * T], MM_DT,
                                       isOutput=False)
    xt_oth = nc.declare_dram_parameter("xt_oth", [HID, SPC * T], MM_DT,
                                       isOutput=False)
    w_in = nc.declare_dram_parameter("w_in", [HID, PROJ], MM_DT, isOutput=False)
    b_in = nc.declare_dram_parameter("b_in", [1, PROJ], F32, isOutput=False)
    br_in = nc.declare_dram_parameter("br_in", [SPC, 3], I32, isOutput=False)
    g1idx = nc.declare_dram_parameter("g1idx", [SPC, 1], I32, isOutput=False)

    emb = nc.declare_dram_parameter("emb", [RPC, PROJ], F32, isOutput=True)
    a_out = nc.declare_dram_parameter("a_out", [SPC, PROJ], F32, isOutput=True)
    sc_out = nc.declare_dram_parameter("sc_out", [SPC, 5], F32, isOutput=True)
    # sc_out cols: 0=c0, 1=c1, 2=s(self dist), 3=numer, 4=softplus term

    ntile = RPC // 128  # 50
    GRP = 10            # tiles per rsqrt batch
    with tile.TileContext(nc) as tc:
        with (
            tc.tile_pool(name="singles", bufs=1) as singles,
            tc.tile_pool(name="xtiles", bufs=1) as xtiles,
            tc.tile_pool(name="work", bufs=4) as work,
            tc.tile_pool(name="psum", bufs=8, space="PSUM") as psum_pool,
        ):
            # persistent operands; inputs split over the two HWDGE queues
            xt_sb = []
            for kt in range(2):
                t_c = singles.tile([128, SPC * T], MM_DT, tag=f"xtc{kt}")
                nc.sync.dma_start(out=t_c, in_=xt_cur[kt * 128:(kt + 1) * 128, :])
                t_o = singles.tile([128, SPC * T], MM_DT, tag=f"xto{kt}")
                nc.scalar.dma_start(out=t_o,
                                    in_=xt_oth[kt * 128:(kt + 1) * 128, :])
                xt_sb.append((t_c, t_o))
            w_sb = []
            for kt in range(2):
                t_w = singles.tile([128, PROJ], MM_DT, tag=f"w{kt}")
                nc.sync.dma_start(out=t_w, in_=w_in[kt * 128:(kt + 1) * 128, :])
                w_sb.append(t_w)
            bias_sb = singles.tile([128, PROJ], F32, tag="bias")
            nc.sync.dma_start(out=bias_sb, in_=b_in[:, :].to_broadcast([128, PROJ]))
            delta_sb = singles.tile([128, 1], F32, tag="delta")
            nc.vector.memset(delta_sb, DELTA)

            mv_all = singles.tile([128, ntile, 2], F32, tag="mv")
            nn_all = singles.tile([128, ntile], F32, tag="nn")
            sn_all = singles.tile([128, ntile], F32, tag="sn")
            rs_all = singles.tile([128, ntile], F32, tag="rs")

            # phase A per tile: matmul -> bias-add evacuation (frees PSUM
            # fast, keeps PE streaming) -> bn_stats norm statistics.
            # Every GRP tiles: batched nn/sqrt/recip, then the scaled+cast
            # stores for that group (overlaps the next group's phase A).
            x_sb = []
            for m in range(ntile):
                half = 0 if m < ntile // 2 else 1
                roff = (m - half * (ntile // 2)) * 128
                ps = psum_pool.tile([128, PROJ], F32)
                for kt in range(2):
                    nc.tensor.matmul(
                        out=ps,
                        lhsT=xt_sb[kt][half][:, roff:roff + 128],
                        rhs=w_sb[kt],
                        start=(kt == 0),
                        stop=(kt == 1),
                    )
                xs = xtiles.tile([128, PROJ], F32, tag=f"x{m}")
                nc.vector.scalar_tensor_tensor(out=xs, in0=ps, scalar=1.0,
                                               in1=bias_sb, op0=OP.mult,
                                               op1=OP.add)
                x_sb.append(xs)
                stats = work.tile([128, 6], F32, tag=f"stats{m % 2}")
                nc.vector.bn_stats(out=stats, in_=xs)
                nc.vector.bn_aggr(out=mv_all[:, m, :], in_=stats)

                if m % GRP == GRP - 1:
                    g0 = m - GRP + 1
                    sl = slice(g0, m + 1)
                    nc.vector.tensor_tensor(
                        out=nn_all[:, sl].unsqueeze(-1),
                        in0=mv_all[:, sl, 0:1], in1=mv_all[:, sl, 0:1],
                        op=OP.mult)
                    nc.vector.tensor_tensor(
                        out=nn_all[:, sl].unsqueeze(-1),
                        in0=nn_all[:, sl].unsqueeze(-1),
                        in1=mv_all[:, sl, 1:2], op=OP.add)
                    nc.scalar.activation(out=sn_all[:, sl], in_=nn_all[:, sl],
                                         func=AF.Sqrt, scale=float(PROJ))
                    nc.vector.reciprocal(out=rs_all[:, sl], in_=sn_all[:, sl])
                    for mm in range(g0, m + 1):
                        ys = work.tile([128, PROJ], F32, tag="y")
                        nc.scalar.activation(out=ys, in_=x_sb[mm],
                                             func=AF.Copy,
                                             scale=rs_all[:, mm:mm + 1])
                        eng = nc.sync if mm % 2 == 0 else nc.gpsimd
                        eng.dma_start(out=emb[mm * 128:(mm + 1) * 128, :],
                                      in_=ys)

            # per-sequence epilogue (reads normalized embeddings back)
            emb_seq = emb[:, :].rearrange("(s t) c -> s t c", t=T)
            for (s0, psz) in _seq_tiles():
                g0 = work.tile([128, PROJ], F32, tag="g0")
                g2 = work.tile([128, PROJ], F32, tag="g2")
                g1 = work.tile([128, PROJ], F32, tag="g1")
                nc.sync.dma_start(out=g0[:psz], in_=emb_seq[s0:s0 + psz, 0, :])
                nc.sync.dma_start(out=g2[:psz], in_=emb_seq[s0:s0 + psz, T - 1, :])
                idx = work.tile([128, 1], I32, tag="idx")
                nc.sync.dma_start(out=idx[:psz], in_=g1idx[s0:s0 + psz, :])
                nc.gpsimd.indirect_dma_start(
                    out=g1[:psz],
                    out_offset=None,
                    in_=emb[:, :],
                    in_offset=bass.IndirectOffsetOnAxis(ap=idx[:psz, :1], axis=0),
                )

                bi = work.tile([128, 3], I32, tag="bi")
                nc.sync.dma_start(out=bi[:psz], in_=br_in[s0:s0 + psz, :])
                bf = work.tile([128, 3], F32, tag="bf")
                nc.vector.tensor_copy(out=bf[:psz], in_=bi[:psz])
                bh, bp, bt = bf[:psz, 0:1], bf[:psz, 1:2], bf[:psz, 2:3]

                def tt(o, i0, i1, op):
                    nc.vector.tensor_tensor(out=o, in0=i0, in1=i1, op=op)

                sc = work.tile([128, 16], F32, tag="sc")
                alpha = sc[:psz, 0:1]
                d2 = sc[:psz, 1:2]
                sig2 = sc[:psz, 2:3]
                rsg = sc[:psz, 3:4]
                q = sc[:psz, 4:5]
                aa = sc[:psz, 5:6]
                score = sc[:psz, 6:7]
                tmp = sc[:psz, 7:8]
                oma = sc[:psz, 8:9]
                c0 = sc[:psz, 9:10]
                s_sd = sc[:psz, 10:11]
                numer = sc[:psz, 11:12]
                sp = sc[:psz, 12:13]
                sig = sc[:psz, 13:14]

                tt(alpha, bp, bh, OP.subtract)          # bp - bh
                tt(d2, bt, bh, OP.subtract)             # bt - bh
                nc.vector.reciprocal(out=d2, in_=d2)
                tt(alpha, alpha, d2, OP.mult)           # alpha
                tt(sig, bt, bp, OP.subtract)            # bt - bp
                tt(sig, alpha, sig, OP.mult)            # sigma
                tt(sig2, sig, sig, OP.mult)             # sigma^2
                nc.vector.reciprocal(out=rsg, in_=sig2)  # 1/sigma^2 == c1

                a_t = work.tile([128, PROJ], F32, tag="a")
                nc.vector.tensor_scalar(out=oma, in0=alpha, scalar1=-1.0,
                                        scalar2=1.0, op0=OP.mult, op1=OP.add)
                nc.vector.tensor_scalar(out=a_t[:psz], in0=g0[:psz], scalar1=oma,
                                        scalar2=None, op0=OP.mult)
                prod = work.tile([128, PROJ], F32, tag="prod")
                nc.vector.tensor_scalar(out=prod[:psz], in0=g2[:psz], scalar1=alpha,
                                        scalar2=None, op0=OP.mult)
                tt(a_t[:psz], a_t[:psz], prod[:psz], OP.add)

                # q = a.g1 ; aa = a.a ; score = g0.g2
                nc.vector.scalar_tensor_tensor(
                    out=prod[:psz], in0=a_t[:psz], scalar=1.0, in1=g1[:psz],
                    op0=OP.mult, op1=OP.mult, accum_out=q)
                nc.vector.scalar_tensor_tensor(
                    out=prod[:psz], in0=a_t[:psz], scalar=1.0, in1=a_t[:psz],
                    op0=OP.mult, op1=OP.mult, accum_out=aa)
                nc.vector.scalar_tensor_tensor(
                    out=prod[:psz], in0=g0[:psz], scalar=1.0, in1=g2[:psz],
                    op0=OP.mult, op1=OP.mult, accum_out=score)

                # s = -(1 - 2q + aa) / (2 sigma^2); numer = exp(s)
                nc.vector.tensor_scalar(out=tmp, in0=q, scalar1=-2.0, scalar2=1.0,
                                        op0=OP.mult, op1=OP.add)
                tt(tmp, tmp, aa, OP.add)
                nc.vector.tensor_scalar(out=tmp, in0=tmp, scalar1=rsg,
                                        scalar2=-0.5, op0=OP.mult, op1=OP.mult)
                nc.vector.tensor_copy(out=s_sd, in_=tmp)
                nc.scalar.activation(out=numer, in_=s_sd, func=AF.Exp)

                # c0 = -(1 + aa) / (2 sigma^2)
                nc.vector.tensor_scalar(out=tmp, in0=aa, scalar1=1.0, scalar2=None,
                                        op0=OP.add)
                nc.vector.tensor_scalar(out=c0, in0=tmp, scalar1=rsg, scalar2=-0.5,
                                        op0=OP.mult, op1=OP.mult)

                # softplus(delta - score) = ln(1 + exp(delta - score))
                nc.scalar.activation(out=tmp, in_=score, func=AF.Exp,
                                     bias=delta_sb[:psz], scale=-1.0)
                nc.scalar.activation(out=sp, in_=tmp, func=AF.Ln, bias=1.0)

                nc.sync.dma_start(out=a_out[s0:s0 + psz, :], in_=a_t[:psz])
                # scalars out: one small column DMA per quantity
                for col_i, col in ((0, c0), (1, rsg), (2, s_sd), (3, numer),
                                   (4, sp)):
                    nc.sync.dma_start(
                        out=sc_out[s0:s0 + psz, col_i:col_i + 1], in_=col)
    nc.compile()
    return nc


def _build_l2():
    nc = bacc.Bacc("TRN2", target_bir_lowering=False, debug=False,
                   num_devices=NCORES)
    a_in = nc.declare_dram_parameter("a_in", [HID, SLOTS], MM_DT, isOutput=False)
    pool_in = nc.declare_dram_parameter("pool_in", [HID, NG, NCOL], MM_DT,
                                        isOutput=False)
    MT = SLOTS // 128  # 28 M-tiles
    top8 = nc.declare_dram_parameter("top8", [128, MT, 8], F32, isOutput=True)

    with tile.TileContext(nc) as tc:
        with (
            tc.tile_pool(name="singles", bufs=1) as singles,
            tc.tile_pool(name="psum", bufs=8, space="PSUM") as psum_pool,
        ):
            # per-group/chunk tiles, loads alternating over the two HWDGE
            # queues so matmuls start as soon as their group's data lands
            engs = (nc.sync, nc.scalar)
            NCH = 7          # a chunks of 4 M-tiles each
            csz = SLOTS // NCH
            a_sb = [[None] * NCH for _ in range(2)]
            pool_sb = [[None] * NG for _ in range(2)]
            ei = 0
            for c in range(NCH):
                for kt in range(2):
                    t_a = singles.tile([128, csz], MM_DT, tag=f"a{kt}_{c}")
                    engs[ei % 2].dma_start(
                        out=t_a,
                        in_=a_in[kt * 128:(kt + 1) * 128,
                                 c * csz:(c + 1) * csz])
                    a_sb[kt][c] = t_a
                    ei += 1
                # interleave the pool groups this a-chunk's M-tiles need
                for g in range(2 * c, min(2 * c + 2, NG)):
                    for kt in range(2):
                        t_p = singles.tile([128, NCOL], MM_DT, tag=f"p{kt}_{g}")
                        engs[ei % 2].dma_start(
                            out=t_p,
                            in_=pool_in[kt * 128:(kt + 1) * 128, g, :])
                        pool_sb[kt][g] = t_p
                        ei += 1

            t8_all = singles.tile([128, MT, 8], F32, tag="t8all")
            mpc = MT // NCH  # M-tiles per a-chunk (4)
            for m in range(MT):
                g = m // (GCAP // 128)     # pivot group, pivot = g+1
                ps = psum_pool.tile([128, NCOL], F32)
                for kt in range(2):
                    nc.tensor.matmul(
                        out=ps,
                        lhsT=a_sb[kt][m // mpc][:, (m % mpc) * 128:
                                                (m % mpc + 1) * 128],
                        rhs=pool_sb[kt][g],
                        start=(kt == 0),
                        stop=(kt == 1),
                    )
                nc.vector.max(out=t8_all[:, m, :], in_=ps)
            nc.gpsimd.dma_start(out=top8[:, :, :], in_=t8_all)
    nc.compile()
    return nc


def _build_l3():
    nc = bacc.Bacc("TRN2", target_bir_lowering=False, debug=False,
                   num_devices=NCORES)
    top64 = nc.declare_dram_parameter("top64", [128, ROWT3, 64], F32,
                                      isOutput=False)
    scal = nc.declare_dram_parameter("scal", [5, 128, ROWT3], F32,
                                     isOutput=False)
    # scal rows: 0=c0, 1=c1, 2=s, 3=numer, 4=sp
    out2 = nc.declare_dram_parameter("out2", [1, 2], F32, isOutput=True)

    with tile.TileContext(nc) as tc:
        with (
            tc.tile_pool(name="singles", bufs=1) as singles,
            tc.tile_pool(name="work", bufs=3) as work,
            tc.tile_pool(name="psum", bufs=2, space="PSUM") as psum_pool,
        ):
            t64 = singles.tile([128, ROWT3, 64], F32, tag="t64")
            nc.sync.dma_start(out=t64, in_=top64[:, :, :])
            sc_c0 = singles.tile([128, ROWT3], F32, tag="scc0")
            sc_c1 = singles.tile([128, ROWT3], F32, tag="scc1")
            sc_s = singles.tile([128, ROWT3], F32, tag="scs")
            sc_nm = singles.tile([128, ROWT3], F32, tag="scnm")
            sc_sp = singles.tile([128, ROWT3], F32, tag="scsp")
            for t_sb, row in ((sc_c0, 0), (sc_c1, 1), (sc_s, 2), (sc_nm, 3),
                              (sc_sp, 4)):
                nc.sync.dma_start(out=t_sb, in_=scal[row, :, :])

            # dist = c1*cross + c0, batched over all 13 row tiles with
            # stride-0 broadcast of the per-row coefficients along the 64
            # candidates
            R = ROWT3
            d64 = singles.tile([128, R, 64], F32, tag="d64")
            c1b = sc_c1[:, :].unsqueeze(-1).to_broadcast([128, R, 64])
            c0b = sc_c0[:, :].unsqueeze(-1).to_broadcast([128, R, 64])
            nc.vector.tensor_tensor(out=d64, in0=t64, in1=c1b, op=OP.mult)
            nc.vector.tensor_tensor(out=d64, in0=d64, in1=c0b, op=OP.add)
            t8a = singles.tile([128, R, 8], F32, tag="t8a")
            for t in range(R):
                nc.vector.max(out=t8a[:, t, :], in_=d64[:, t, :])
            # exp of the top-6 of each tile, summed per tile
            e6 = singles.tile([128, R, 6], F32, tag="e6")
            nc.scalar.activation(out=e6, in_=t8a[:, :, 0:6], func=AF.Exp)
            se6 = singles.tile([128, R], F32, tag="se6")
            nc.vector.reduce_sum(out=se6[:, :].unsqueeze(-1), in_=e6,
                                 axis=mybir.AxisListType.X)
            # subtract exp(max(s, v6)); deno = numer + se6 - em
            mx = singles.tile([128, R], F32, tag="mx")
            nc.vector.tensor_tensor(out=mx[:, :].unsqueeze(-1),
                                    in0=t8a[:, :, 5:6],
                                    in1=sc_s[:, :].unsqueeze(-1), op=OP.max)
            em = singles.tile([128, R], F32, tag="em")
            nc.scalar.activation(out=em, in_=mx, func=AF.Exp)
            deno = singles.tile([128, R], F32, tag="deno")
            nc.vector.tensor_tensor(out=deno, in0=se6, in1=em, op=OP.subtract)
            nc.vector.tensor_tensor(out=deno, in0=deno, in1=sc_nm, op=OP.add)
            nc.vector.reciprocal(out=deno, in_=deno)
            loss = singles.tile([128, ROWT3], F32, tag="loss")
            nc.vector.tensor_tensor(out=loss, in0=sc_nm, in1=deno, op=OP.mult)

            ones = singles.tile([128, 2], F32, tag="ones")
            nc.vector.memset(ones, 1.0)
            red = singles.tile([128, 2], F32, tag="red")
            nc.vector.reduce_sum(out=red[:, 0:1], in_=loss, axis=mybir.AxisListType.X)
            nc.vector.reduce_sum(out=red[:, 1:2], in_=sc_sp, axis=mybir.AxisListType.X)
            ps = psum_pool.tile([1, 2], F32)
            nc.tensor.matmul(out=ps, lhsT=ones[:, 0:1], rhs=red,
                             start=True, stop=True)
            fin = singles.tile([1, 2], F32, tag="fin")
            nc.vector.tensor_scalar(out=fin, in0=ps, scalar1=1.0 / NSEQ,
                                    scalar2=None, op0=OP.mult)
            nc.sync.dma_start(out=out2[:, :], in_=fin)
    nc.compile()
    return nc


_NC_CACHE = {}


def _get(name, builder):
    if name not in _NC_CACHE:
        _NC_CACHE[name] = builder()
    return _NC_CACHE[name]


LAST_RUNS = []


def _hw_runner(nc, in_maps):
    import os
    res = run_bass_kernel_spmd(
        nc, in_maps, list(range(NCORES)),
        trace=bool(os.environ.get("KERNEL_TRACE")))
    LAST_RUNS.append(res)
    return res.results


def kernel(frame_embeds, other_frame_embeds, W, b, bridge, _runner=None):
    frame_embeds = np.asarray(frame_embeds, dtype=np.float32)
    other_frame_embeds = np.asarray(other_frame_embeds, dtype=np.float32)
    W = np.asarray(W, dtype=np.float32)
    b = np.asarray(b, dtype=np.float32)
    bridge = np.asarray(bridge, dtype=np.int32)

    runner = _runner if _runner is not None else _hw_runner

    # ---- host-side sharding / layout (pure indexing) ----
    fe_seq = frame_embeds.transpose(0, 2, 1, 3).reshape(NSEQ, T, HID)
    ofe_seq = other_frame_embeds.transpose(0, 2, 1, 3).reshape(NSEQ, T, HID)
    perm = np.argsort(bridge[:, 1], kind="stable")
    fe_sorted = fe_seq[perm]
    bridge_s = bridge[perm]

    b_in = np.ascontiguousarray(b.reshape(1, PROJ))
    in1 = []
    for k in range(NCORES):
        sl = slice(k * SPC, (k + 1) * SPC)
        xt_cur = np.ascontiguousarray(fe_sorted[sl].reshape(SPC * T, HID).T)
        xt_oth = np.ascontiguousarray(ofe_seq[sl].reshape(SPC * T, HID).T)
        br_k = np.ascontiguousarray(bridge_s[sl])
        g1i = (np.arange(SPC, dtype=np.int32) * T
               + br_k[:, 1].astype(np.int32)).reshape(SPC, 1)
        in1.append({"xt_cur": xt_cur, "xt_oth": xt_oth, "w_in": W,
                    "b_in": b_in, "br_in": br_k, "g1idx": g1i})

    nc1 = _get("l1", _build_l1)
    r1 = runner(nc1, in1)

    # ---- host reshard between L1 and L2 (pure indexing) ----
    a_all = np.concatenate([r1[k]["a_out"] for k in range(NCORES)], axis=0)
    sc_all = np.concatenate([r1[k]["sc_out"] for k in range(NCORES)], axis=0).T
    # sc_all: (5, 1600) rows c0, c1, s, numer, sp

    piv = bridge_s[:, 1].astype(np.int64)  # sorted ascending, values 1..14
    counts = np.bincount(piv, minlength=T)[1:T - 1]
    assert counts.max() <= GCAP, f"pivot group overflow: {counts.max()} > {GCAP}"
    gstart = np.zeros(NG, dtype=np.int64)
    gstart[1:] = np.cumsum(counts)[:-1]
    rank = np.arange(NSEQ, dtype=np.int64) - gstart[piv - 1]
    slot_of = (piv - 1) * GCAP + rank  # slot per sorted row

    a_pad = np.zeros((HID, SLOTS), dtype=np.float32)
    a_pad[:, slot_of] = a_all.T

    in2 = []
    for k in range(NCORES):
        emb_k = r1[k]["emb"].reshape(2, SPC, T, HID)
        both = np.concatenate([emb_k[0][:, 1:T - 1, :],
                               emb_k[1][:, 1:T - 1, :]], axis=0)  # (400,14,256)
        pool_k = np.ascontiguousarray(both.transpose(2, 1, 0))    # (256,14,400)
        in2.append({"a_in": a_pad, "pool_in": pool_k})

    nc2 = _get("l2", _build_l2)
    r2 = runner(nc2, in2)

    # ---- host gather for L3 (pure indexing) ----
    top64 = np.concatenate(
        [r2[k]["top8"].transpose(1, 0, 2).reshape(SLOTS, 8)[slot_of]
         for k in range(NCORES)], axis=1)  # (1600, 64)
    top64_p = np.zeros((NPAD3 , 64), dtype=np.float32)
    top64_p[:NSEQ] = top64
    t64_in = np.ascontiguousarray(
        top64_p.reshape(ROWT3, 128, 64).transpose(1, 0, 2))

    scal_p = np.zeros((5, NPAD3), dtype=np.float32)
    scal_p[:, :NSEQ] = sc_all
    scal_in = np.ascontiguousarray(
        scal_p.reshape(5, ROWT3, 128).transpose(0, 2, 1))

    in3 = [{"top64": t64_in, "scal": scal_in} for _ in range(NCORES)]
    nc3 = _get("l3", _build_l3)
    r3 = runner(nc3, in3)

    out = r3[0]["out2"]
    brownian_loss = np.float32(out[0, 0])
    head_tail_match = np.float32(out[0, 1])
    return (np.asarray(brownian_loss), np.asarray(head_tail_match))

